# revision 2
# baseline (speedup 1.0000x reference)
"""Trainium2 Bass kernel for nn_CINLayer (3-layer CIN: chained bilinear einsums).

Strategy (data-parallel over batch, 8 cores x 512 rows):
  X1 = einsum('hjk,bjd,bkd->bhd', W0r, X0, X0); S1 = X1.sum(d)
  X2 = einsum(W1r, X0, X1);                     S2 = X2.sum(d)
  S3 = einsum over the Gram matrix G[b,j,k] = sum_d X0[b,j,d] X2[b,k,d]
       (final layer output only needs the d-sum, so X3 is never materialized)

Device layout: "c-major" Khatri-Rao product tiles P[(j,k), n] with n=(b,d),
built by DVE tensor-tensor multiplies against partition-broadcast rows of X0
(broadcast done by DMA from DRAM with a stride-0 source AP), consumed by the
PE as accumulating matmuls. The last layer uses per-8-batch Gram matmuls with
a block-diagonal host-built rhs.
"""

import sys

import numpy as np

try:
    import concourse.bass as bass  # noqa: F401
except ImportError:
    sys.path.insert(0, "/opt/trn_rl_repo")

import ml_dtypes

BF16 = ml_dtypes.bfloat16

B, F0, D, H = 4096, 39, 16, 128
N_CORES = 8
BC = B // N_CORES            # 512 batch rows per core
N = BC * D                   # 8192 columns, n = (b, d), d innermost
NH = N // 2                  # half width for PSUM capacity (8 banks x 512)
KP = 64                      # layer-0 k padded 39 -> 64
C0_CHUNKS = 20               # ceil(40*64 / 128): j-pairs
C1_CHUNKS = 39               # 39 j's, k = 128 dense
NT8 = BC // 8                # 64 tiles of 8 batch rows (Gram)
GQ = 4                       # process Gram/S3 in 4 quarters of 128 b

_CACHE = {}


def _build():
    import concourse.bass as bass
    import concourse.tile as tile
    from concourse import bacc, mybir

    bf16 = mybir.dt.bfloat16
    f32 = mybir.dt.float32
    AF = mybir.ActivationFunctionType
    AX = mybir.AxisListType

    nc = bacc.Bacc("TRN2", target_bir_lowering=False, debug=False,
                   num_devices=N_CORES)

    x0t_d = nc.dram_tensor("x0t", [F0 + 1, N], bf16, kind="ExternalInput")
    x0stack_d = nc.dram_tensor("x0stack", [128, N], bf16, kind="ExternalInput")
    w0_d = nc.dram_tensor("w0", [128, C0_CHUNKS, 128], bf16, kind="ExternalInput")
    w1_d = nc.dram_tensor("w1", [128, C1_CHUNKS, 128], bf16, kind="ExternalInput")
    w2_d = nc.dram_tensor("w2", [128, C1_CHUNKS, 128], bf16, kind="ExternalInput")
    x0bd_d = nc.dram_tensor("x0bd", [128, NT8 * 312], bf16, kind="ExternalInput")
    b0_d = nc.dram_tensor("b0", [128, 1], f32, kind="ExternalInput")
    b1_d = nc.dram_tensor("b1", [128, 1], f32, kind="ExternalInput")
    s1_d = nc.dram_tensor("s1", [128, BC], f32, kind="ExternalOutput")
    s2_d = nc.dram_tensor("s2", [128, BC], f32, kind="ExternalOutput")
    s3_d = nc.dram_tensor("s3", [128, BC], f32, kind="ExternalOutput")

    from contextlib import ExitStack

    with tile.TileContext(nc) as tc, ExitStack() as ctx:
        const = ctx.enter_context(tc.tile_pool(name="const", bufs=1))
        bcpool = ctx.enter_context(tc.tile_pool(name="bc", bufs=3))
        ppool = ctx.enter_context(tc.tile_pool(name="pp", bufs=3))
        x2dtpool = ctx.enter_context(tc.tile_pool(name="x2dtp", bufs=2))
        x0bdpool = ctx.enter_context(tc.tile_pool(name="x0bdp", bufs=3))
        gpool = ctx.enter_context(tc.tile_pool(name="gp", bufs=2))

        x0stack_t = const.tile([128, N], bf16)
        nc.sync.dma_start(out=x0stack_t[:], in_=x0stack_d.ap())
        w0_t = const.tile([128, C0_CHUNKS, 128], bf16)
        nc.sync.dma_start(out=w0_t[:], in_=w0_d.ap())
        w1_t = const.tile([128, C1_CHUNKS, 128], bf16)
        nc.sync.dma_start(out=w1_t[:], in_=w1_d.ap())
        w2_t = const.tile([128, C1_CHUNKS, 128], bf16)
        nc.sync.dma_start(out=w2_t[:], in_=w2_d.ap())
        b0_t = const.tile([128, 1], f32)
        nc.sync.dma_start(out=b0_t[:], in_=b0_d.ap())
        b1_t = const.tile([128, 1], f32)
        nc.sync.dma_start(out=b1_t[:], in_=b1_d.ap())

        x1_t = const.tile([128, N], bf16)
        x2_t = const.tile([128, N], bf16)
        s1_sb = const.tile([128, BC], f32)
        s2_sb = const.tile([128, BC], f32)
        s3_sb = const.tile([128, BC], f32)

        layers = [
            (C0_CHUNKS, w0_t, x0stack_t, b0_t, x1_t, s1_sb),
            (C1_CHUNKS, w1_t, x1_t, b1_t, x2_t, s2_sb),
        ]

        with tc.tile_pool(name="psumA", bufs=8, space="PSUM") as psumA:
            for li, (nchunks, w_t, in0_t, bias_t, xout_t, ssb_t) in enumerate(layers):
                for half in range(2):
                    c0 = half * NH
                    acc = [psumA.tile([128, 512], f32, tag="acc",
                                      name=f"acc_{li}_{half}_{t}")
                           for t in range(8)]
                    for c in range(nchunks):
                        in1 = bcpool.tile([128, NH], bf16, tag="bc")
                        if li == 0:
                            # paired-row broadcast: rows 2c,2c+1 -> partitions 0-63, 64-127
                            src = bass.AP(x0t_d, (2 * c) * N + c0,
                                          [[N, 2], [0, 64], [1, NH]])
                        else:
                            src = x0t_d.ap()[c:c + 1, c0:c0 + NH].to_broadcast((128, NH))
                        nc.sync.dma_start(out=in1[:], in_=src)
                        p = ppool.tile([128, NH], bf16, tag="p")
                        nc.vector.tensor_mul(p[:], in0_t[:, c0:c0 + NH], in1[:])
                        for t in range(8):
                            nc.tensor.matmul(acc[t][:], lhsT=w_t[:, c, :],
                                             rhs=p[:, t * 512:(t + 1) * 512],
                                             start=(c == 0), stop=(c == nchunks - 1))
                    for t in range(8):
                        nc.scalar.activation(
                            xout_t[:, c0 + t * 512: c0 + (t + 1) * 512], acc[t][:],
                            AF.Identity, bias=bias_t[:], scale=1.0)
                        nc.vector.reduce_sum(
                            ssb_t[:, half * (NH // D) + t * 32:
                                  half * (NH // D) + (t + 1) * 32],
                            acc[t][:].rearrange("p (b d) -> p b d", d=D),
                            axis=AX.X)

        with tc.tile_pool(name="psumB", bufs=2, space="PSUM") as psumB:
            for q in range(GQ):
                x2dt = x2dtpool.tile([128, NT8 // GQ, 128], bf16, tag="x2dt")
                g = gpool.tile([128, F0, 128], bf16, tag="g")
                for t16 in range(NT8 // GQ):
                    t = q * (NT8 // GQ) + t16
                    nc.scalar.dma_start_transpose(
                        out=x2dt[:, t16, :], in_=x2_t[:, t * 128:(t + 1) * 128])
                    x0bd = x0bdpool.tile([128, 312], bf16, tag="x0bd")
                    nc.scalar.dma_start(out=x0bd[:],
                                        in_=x0bd_d.ap()[:, t * 312:(t + 1) * 312])
                    psg = psumB.tile([128, 312], f32, tag="gram")
                    nc.tensor.matmul(psg[:], lhsT=x2dt[:, t16, :], rhs=x0bd[:],
                                     start=True, stop=True)
                    nc.scalar.activation(
                        g[:, :, t16 * 8:(t16 + 1) * 8],
                        psg[:].rearrange("p (b j) -> p j b", b=8),
                        AF.Copy)
                pss3 = psumB.tile([128, 128], f32, tag="s3")
                for j in range(F0):
                    nc.tensor.matmul(pss3[:], lhsT=w2_t[:, j, :], rhs=g[:, j, :],
                                     start=(j == 0), stop=(j == F0 - 1))
                nc.scalar.activation(s3_sb[:, q * 128:(q + 1) * 128], pss3[:], AF.Copy)

        nc.sync.dma_start(out=s1_d.ap(), in_=s1_sb[:])
        nc.sync.dma_start(out=s2_d.ap(), in_=s2_sb[:])
        nc.sync.dma_start(out=s3_d.ap(), in_=s3_sb[:])

    nc.compile()
    return nc


def _prep_core(Xc, w0l, w1l, w2l, b0, b1):
    """Per-core input maps. Xc: [BC, F0, D] float32."""
    x0t_f = Xc.transpose(1, 0, 2).reshape(F0, N)          # [j, (b,d)]
    x0t = np.zeros((F0 + 1, N), dtype=BF16)
    x0t[:F0] = x0t_f.astype(BF16)

    x0stack = np.zeros((128, N), dtype=BF16)
    x0stack[0:F0] = x0t[:F0]
    x0stack[64:64 + F0] = x0t[:F0]

    # block-diagonal Gram rhs: [128=(8b,16d), (t, 8b, 39j)]
    tmp = Xc.reshape(NT8, 8, F0, D).transpose(0, 1, 3, 2)   # [t, bb, d, j]
    arr = np.zeros((NT8, 8, D, 8, F0), dtype=BF16)
    idx = np.arange(8)
    arr[:, idx, :, idx, :] = tmp.transpose(1, 0, 2, 3).astype(BF16)
    x0bd = arr.reshape(NT8, 128, 312).transpose(1, 0, 2).reshape(128, NT8 * 312)
    x0bd = np.ascontiguousarray(x0bd)

    return {
        "x0t": x0t, "x0stack": x0stack,
        "w0": w0l, "w1": w1l, "w2": w2l, "x0bd": x0bd,
        "b0": b0.reshape(128, 1).astype(np.float32),
        "b1": b1.reshape(128, 1).astype(np.float32),
    }


def kernel(embedded_features, W0, b0, W1, b1, W2, b2):
    from concourse.bass_utils import run_bass_kernel_spmd

    X = np.asarray(embedded_features, dtype=np.float32)
    W0 = np.asarray(W0, dtype=np.float32)
    W1 = np.asarray(W1, dtype=np.float32)
    W2 = np.asarray(W2, dtype=np.float32)
    b0 = np.asarray(b0, dtype=np.float32)
    b1 = np.asarray(b1, dtype=np.float32)
    b2 = np.asarray(b2, dtype=np.float32)

    # --- weight layouts (shared across cores) ---
    # L0: k padded to 64, j padded to 40; chunk i = j-pair (2i, 2i+1)
    W0r = W0.reshape(H, F0, F0)
    W0p = np.zeros((H, F0 + 1, KP), dtype=np.float32)
    W0p[:, :F0, :F0] = W0r
    # [h, i, jj, k] -> [kk=(jj,k), i, h]
    w0l = np.ascontiguousarray(
        W0p.reshape(H, C0_CHUNKS, 2, KP).transpose(2, 3, 1, 0)
        .reshape(128, C0_CHUNKS, 128).astype(BF16))
    w1l = np.ascontiguousarray(
        W1.reshape(H, F0, 128).transpose(2, 1, 0).astype(BF16))
    w2l = np.ascontiguousarray(
        W2.reshape(H, F0, 128).transpose(2, 1, 0).astype(BF16))

    if "nc" not in _CACHE:
        _CACHE["nc"] = _build()
    nc = _CACHE["nc"]

    in_maps = [
        _prep_core(X[c * BC:(c + 1) * BC], w0l, w1l, w2l, b0, b1)
        for c in range(N_CORES)
    ]
    res = run_bass_kernel_spmd(nc, in_maps, core_ids=list(range(N_CORES)))

    out = np.empty((B, 3 * H), dtype=np.float32)
    for c in range(N_CORES):
        r = res.results[c]
        sl = slice(c * BC, (c + 1) * BC)
        out[sl, 0:H] = r["s1"].T + D * b0[None, :]
        out[sl, H:2 * H] = r["s2"].T + D * b1[None, :]
        out[sl, 2 * H:3 * H] = r["s3"].T + D * b2[None, :]
    return out


# revision 14
# speedup vs baseline: 3066.0879x; 3066.0879x over previous
"""Trainium2 Bass kernel for nn_CINLayer (3-layer CIN: chained bilinear einsums).

Strategy (data-parallel over batch, 8 cores x 512 rows):
  X1 = einsum('hjk,bjd,bkd->bhd', W0r, X0, X0); S1 = X1.sum(d)
  X2 = einsum(W1r, X0, X1);                     S2 = X2.sum(d)
  S3 = einsum over the Gram matrix G[b,j,k] = sum_d X0[b,j,d] X2[b,k,d]
       (final layer output only needs the d-sum, so X3 is never materialized)

Device layout: "c-major" Khatri-Rao product tiles P[(j,k), n] with n=(b,d),
built by DVE tensor-tensor multiplies against partition-broadcast rows of X0
(broadcast done by DMA from DRAM with stride-0 / row-replicating source APs),
consumed by the PE as accumulating matmuls. L0 packs 3 j's x 39 k per
117-partition chunk (no padding); L1 uses 39 chunks of (1 j x 128 k).
The last layer uses per-8-batch Gram matmuls (lhsT = DMA-transposed X2
tiles) against a block-diagonal host-built X0 rhs. Work proceeds in four
2048-column quarters with the Gram/S3 stage pipelined behind each L1
quarter.
"""

import sys

import numpy as np

try:
    import concourse.bass as bass  # noqa: F401
except ImportError:
    sys.path.insert(0, "/opt/trn_rl_repo")

import ml_dtypes

BF16 = ml_dtypes.bfloat16

B, F0, D, H = 4096, 39, 16, 128
N_CORES = 8
BC = B // N_CORES            # 512 batch rows per core
N = BC * D                   # 8192 columns, n = (b, d), d innermost
NQ = N // 4                  # 2048-column quarters (4 PSUM banks each)
C0_CHUNKS = 13               # j-triples: 3 j x 39 k = 117 rows per chunk
C0_ROWS = 117
C1_CHUNKS = 39               # 39 j's, k = 128 dense
NT8 = BC // 8                # 64 tiles of 8 batch rows (Gram)
GQ = 4                       # Gram/S3 quarters (128 b each), one per n-quarter

_CACHE = {}


def _build():
    import concourse.bass as bass
    import concourse.tile as tile
    from concourse import bacc, mybir

    bf16 = mybir.dt.bfloat16
    f32 = mybir.dt.float32
    AF = mybir.ActivationFunctionType
    AX = mybir.AxisListType

    nc = bacc.Bacc("TRN2", target_bir_lowering=False, debug=False,
                   num_devices=N_CORES)

    x0t_d = nc.dram_tensor("x0t", [F0, N], bf16, kind="ExternalInput")
    x0trip_d = nc.dram_tensor("x0trip", [C0_ROWS, N], bf16, kind="ExternalInput")
    w0_d = nc.dram_tensor("w0", [128, C0_CHUNKS, 128], bf16, kind="ExternalInput")
    w1_d = nc.dram_tensor("w1", [128, C1_CHUNKS, 128], bf16, kind="ExternalInput")
    w2_d = nc.dram_tensor("w2", [128, C1_CHUNKS, 128], bf16, kind="ExternalInput")
    x0bd_d = nc.dram_tensor("x0bd", [128, NT8 * 312], bf16, kind="ExternalInput")
    ones1_d = nc.dram_tensor("ones1", [1, 128], bf16, kind="ExternalInput")
    ones3_d = nc.dram_tensor("ones3", [3, C0_ROWS], bf16, kind="ExternalInput")
    b0_d = nc.dram_tensor("b0", [128, 1], f32, kind="ExternalInput")
    b1_d = nc.dram_tensor("b1", [128, 1], f32, kind="ExternalInput")
    s1_d = nc.dram_tensor("s1", [128, BC], f32, kind="ExternalOutput")
    s2_d = nc.dram_tensor("s2", [128, BC], f32, kind="ExternalOutput")
    s3_d = nc.dram_tensor("s3", [128, BC], f32, kind="ExternalOutput")

    from contextlib import ExitStack

    with tile.TileContext(nc) as tc, ExitStack() as ctx:
        const = ctx.enter_context(tc.tile_pool(name="const", bufs=1))
        bcpool = ctx.enter_context(tc.tile_pool(name="bc", bufs=8))
        ppool = ctx.enter_context(tc.tile_pool(name="pp", bufs=8))
        x2dtpool = ctx.enter_context(tc.tile_pool(name="x2dtp", bufs=2))
        x0bdpool = ctx.enter_context(tc.tile_pool(name="x0bdp", bufs=2))
        gpool = ctx.enter_context(tc.tile_pool(name="gp", bufs=2))
        rowpool = ctx.enter_context(tc.tile_pool(name="rowp", bufs=4))

        x0trip_t = const.tile([C0_ROWS, N], bf16)
        nc.sync.dma_start(out=x0trip_t[:], in_=x0trip_d.ap())
        w0_t = const.tile([128, C0_CHUNKS, 128], bf16)
        nc.sync.dma_start(out=w0_t[:], in_=w0_d.ap())
        w1_t = const.tile([128, C1_CHUNKS, 128], bf16)
        nc.sync.dma_start(out=w1_t[:], in_=w1_d.ap())
        w2_t = const.tile([128, C1_CHUNKS, 128], bf16)
        nc.sync.dma_start(out=w2_t[:], in_=w2_d.ap())
        ones1_t = const.tile([1, 128], bf16)
        nc.sync.dma_start(out=ones1_t[:], in_=ones1_d.ap())
        ones3_t = const.tile([3, C0_ROWS], bf16)
        nc.sync.dma_start(out=ones3_t[:], in_=ones3_d.ap())
        b0_t = const.tile([128, 1], f32)
        nc.sync.dma_start(out=b0_t[:], in_=b0_d.ap())
        b1_t = const.tile([128, 1], f32)
        nc.sync.dma_start(out=b1_t[:], in_=b1_d.ap())

        x1_t = const.tile([128, N], bf16)
        x2_t = const.tile([128, N], bf16)
        s1_sb = const.tile([128, BC], f32)
        s2_sb = const.tile([128, BC], f32)
        s3_sb = const.tile([128, BC], f32)

        layers = [
            (C0_CHUNKS, C0_ROWS, w0_t, x0trip_t, b0_t, x1_t, s1_sb),
            (C1_CHUNKS, 128, w1_t, x1_t, b1_t, x2_t, s2_sb),
        ]

        with tc.tile_pool(name="psum", bufs=1, space="PSUM") as psum:
            for li, (nchunks, nrows, w_t, in0_t, bias_t, xout_t, ssb_t) in \
                    enumerate(layers):
                for q in range(4):
                    c0 = q * NQ
                    acc = [psum.tile([128, 512], f32, tag="acc", bufs=6,
                                     name=f"acc_{li}_{q}_{t}")
                           for t in range(4)]
                    gsz = 1
                    groups = [list(range(s, min(s + gsz, nchunks)))
                              for s in range(0, nchunks, gsz)]
                    for grp in groups:
                        gb = len(grp)
                        in1x = bcpool.tile([nrows, gb, NQ], bf16, tag="bc",
                                           name=f"in1x_{li}_{q}_{grp[0]}")
                        if li == 0:
                            # rows 3c..3c+2 -> partitions 0-38/39-77/78-116
                            src = bass.AP(x0t_d, (3 * grp[0]) * N + c0,
                                          [[N, 3], [0, F0], [1, NQ]])
                        else:
                            src = bass.AP(x0t_d, grp[0] * N + c0,
                                          [[0, 128], [N, gb], [1, NQ]])
                        nc.sync.dma_start(out=in1x[:], in_=src)
                        for ci, c in enumerate(grp):
                            p = ppool.tile([nrows, NQ], bf16, tag="p")
                            nc.vector.tensor_mul(p[:], in0_t[:, c0:c0 + NQ],
                                                 in1x[:, ci, :])
                            for t in range(4):
                                nc.tensor.matmul(acc[t][:],
                                                 lhsT=w_t[0:nrows, c, :],
                                                 rhs=p[:, t * 512:(t + 1) * 512],
                                                 start=(c == 0),
                                                 stop=(c == nchunks - 1))
                    for t in range(4):
                        nc.scalar.activation(
                            xout_t[:, c0 + t * 512: c0 + (t + 1) * 512],
                            acc[t][:], AF.Identity, bias=bias_t[:], scale=1.0)
                        nc.vector.reduce_sum(
                            ssb_t[:, q * 128 + t * 32: q * 128 + (t + 1) * 32],
                            xout_t[:, c0 + t * 512: c0 + (t + 1) * 512]
                            .rearrange("p (b d) -> p b d", d=D),
                            axis=AX.X)

                    if li == 1:
                        # Gram + S3 for this quarter (128 batch rows)
                        x2dt = x2dtpool.tile([128, NT8 // GQ, 128], bf16,
                                             tag="x2dt", name=f"x2dt_{q}")
                        x0bdq = x0bdpool.tile([128, (NT8 // GQ) * 312], bf16,
                                              tag="x0bd", name=f"x0bdq_{q}")
                        nc.sync.dma_start(
                            out=x0bdq[:],
                            in_=x0bd_d.ap()[:, q * (NT8 // GQ) * 312:
                                            (q + 1) * (NT8 // GQ) * 312])
                        g = gpool.tile([128, F0, 128], bf16, tag="g",
                                       name=f"g_{q}")
                        for t16 in range(NT8 // GQ):
                            t = q * (NT8 // GQ) + t16
                            nc.sync.dma_start_transpose(
                                out=x2dt[:, t16, :],
                                in_=x2_t[:, t * 128:(t + 1) * 128])

                            psg = psum.tile([128, 312], f32, tag="gram", bufs=2,
                                            name=f"psg_{q}_{t16}")
                            nc.tensor.matmul(psg[:], lhsT=x2dt[:, t16, :],
                                             rhs=x0bdq[:, t16 * 312:(t16 + 1) * 312],
                                             start=True, stop=True)
                            nc.scalar.activation(
                                g[:, :, t16 * 8:(t16 + 1) * 8],
                                psg[:].rearrange("p (b j) -> p j b", b=8),
                                AF.Copy)
                        pss3 = psum.tile([128, 128], f32, tag="acc", bufs=6,
                                         name=f"pss3_{q}")
                        for j in range(F0):
                            nc.tensor.matmul(pss3[:], lhsT=w2_t[:, j, :],
                                             rhs=g[:, j, :],
                                             start=(j == 0), stop=(j == F0 - 1))
                        nc.scalar.activation(s3_sb[:, q * 128:(q + 1) * 128],
                                             pss3[:], AF.Copy)

        nc.sync.dma_start(out=s1_d.ap(), in_=s1_sb[:])
        nc.sync.dma_start(out=s2_d.ap(), in_=s2_sb[:])
        nc.sync.dma_start(out=s3_d.ap(), in_=s3_sb[:])

    nc.compile()
    return nc


def _prep_core(Xc, w0l, w1l, w2l, b0, b1):
    """Per-core input maps. Xc: [BC, F0, D] float32."""
    x0t = Xc.transpose(1, 0, 2).reshape(F0, N).astype(BF16)   # [j, (b,d)]
    x0trip = np.ascontiguousarray(np.tile(x0t, (3, 1)))       # [117, N]

    # block-diagonal Gram rhs: [128=(8b,16d), (t, 8b, 39j)]
    tmp = Xc.reshape(NT8, 8, F0, D).transpose(0, 1, 3, 2)     # [t, bb, d, j]
    arr = np.zeros((NT8, 8, D, 8, F0), dtype=BF16)
    idx = np.arange(8)
    arr[:, idx, :, idx, :] = tmp.transpose(1, 0, 2, 3).astype(BF16)
    x0bd = arr.reshape(NT8, 128, 312).transpose(1, 0, 2).reshape(128, NT8 * 312)
    x0bd = np.ascontiguousarray(x0bd)

    ones1 = np.ones((1, 128), dtype=BF16)
    ones3 = np.zeros((3, C0_ROWS), dtype=BF16)
    for r in range(3):
        ones3[r, r * F0:(r + 1) * F0] = 1
    return {
        "x0t": x0t, "x0trip": x0trip, "ones1": ones1, "ones3": ones3,
        "w0": w0l, "w1": w1l, "w2": w2l, "x0bd": x0bd,
        "b0": b0.reshape(128, 1).astype(np.float32),
        "b1": b1.reshape(128, 1).astype(np.float32),
    }


def _prep_weights(W0, W1, W2):
    # L0: chunk i = j-triple (3i, 3i+1, 3i+2), rows kk = (jj, k) in 3x39
    W0r = W0.reshape(H, F0, F0)
    w0l = np.zeros((128, C0_CHUNKS, 128), dtype=BF16)
    # [h, i, jj, k] -> [kk=(jj,k), i, h]
    w0l[:C0_ROWS] = (W0r.reshape(H, C0_CHUNKS, 3, F0)
                     .transpose(2, 3, 1, 0).reshape(C0_ROWS, C0_CHUNKS, H)
                     .astype(BF16))
    w1l = np.ascontiguousarray(
        W1.reshape(H, F0, 128).transpose(2, 1, 0).astype(BF16))
    w2l = np.ascontiguousarray(
        W2.reshape(H, F0, 128).transpose(2, 1, 0).astype(BF16))
    return w0l, w1l, w2l


def kernel(embedded_features, W0, b0, W1, b1, W2, b2):
    from concourse.bass_utils import run_bass_kernel_spmd

    X = np.asarray(embedded_features, dtype=np.float32)
    b0 = np.asarray(b0, dtype=np.float32)
    b1 = np.asarray(b1, dtype=np.float32)
    b2 = np.asarray(b2, dtype=np.float32)
    w0l, w1l, w2l = _prep_weights(np.asarray(W0, dtype=np.float32),
                                  np.asarray(W1, dtype=np.float32),
                                  np.asarray(W2, dtype=np.float32))

    if "nc" not in _CACHE:
        _CACHE["nc"] = _build()
    nc = _CACHE["nc"]

    in_maps = [
        _prep_core(X[c * BC:(c + 1) * BC], w0l, w1l, w2l, b0, b1)
        for c in range(N_CORES)
    ]
    res = run_bass_kernel_spmd(nc, in_maps, core_ids=list(range(N_CORES)))

    out = np.empty((B, 3 * H), dtype=np.float32)
    for c in range(N_CORES):
        r = res.results[c]
        sl = slice(c * BC, (c + 1) * BC)
        out[sl, 0:H] = r["s1"].T + D * b0[None, :]
        out[sl, H:2 * H] = r["s2"].T + D * b1[None, :]
        out[sl, 2 * H:3 * H] = r["s3"].T + D * b2[None, :]
    return out
